# revision 3
# baseline (speedup 1.0000x reference)
"""Bahdanau attention on 8 Trainium2 NeuronCores.

Problem (full shapes): hidden [32,1024], encoder_outputs [2048,32,2048],
Wa [1024,1024], Ua [1024,2048], Va [1,1024].

reference:
    enc    = encoder_outputs.transpose(1,0,2)        # [B,S,2H]
    h_proj = hidden @ Wa.T                           # [B,H]
    e_proj = einsum('bsd,hd->bsh', enc, Ua)          # [B,S,H]
    energy = relu(h_proj[:,None,:] + e_proj)         # [B,S,H]
    scores = einsum('bsh,h->bs', energy, Va[0])      # [B,S]
    attn   = softmax(scores, axis=-1)                # [B,S]
    ctx    = einsum('bs,bsd->bd', attn, enc)[:,None] # [B,1,2H]
    return (ctx, attn)

Strategy: data-parallel over batch (4 batches/core).  Per batch the
dominant matmul is enc_b @ Ua.T ([S,2H]x[2H,H]); computed transposed as
e_projT[h,s] so that the relu+h_proj bias is a per-partition scalar on
the scalar engine and the Va contraction is a K=H matmul.  Context is a
K=S matmul with attn (PE-transposed to a column) as the stationary
operand.  Host pre-transposes encoder_outputs so the contraction dims
land on SBUF partitions.
"""

import os
import sys

sys.path.insert(0, "/opt/trn_rl_repo")

import numpy as np
import ml_dtypes

import concourse.bacc as bacc
import concourse.tile as tile
import concourse.mybir as mybir
from concourse import bass_utils

B, S, H = 32, 2048, 1024
D = 2 * H
NCORES = 8
BL = B // NCORES  # batches per core
P = 128
NCH = 512          # matmul moving free-dim chunk (one PSUM bank)
KD = D // P        # 16 contraction tiles over D (e_proj)
KH = H // P        # 8 contraction tiles over H (scores)
KS = S // P        # 16 contraction tiles over S (context)
NS = S // NCH      # 4 s-chunks
ND = D // NCH      # 4 d-chunks

# "bf16" | "f32" | "f32r"
MM_DT = os.environ.get("BAH_MM_DT", "bf16")

_CACHE = {}


def _build(mm_dt: str):
    f32 = mybir.dt.float32
    if mm_dt == "bf16":
        mdt = f32  # placeholder, replaced below
        mdt = mybir.dt.bfloat16

        def bc(ap):
            return ap
    elif mm_dt == "f32r":
        mdt = f32

        def bc(ap):
            return ap.bitcast(mybir.dt.float32r)
    else:
        mdt = f32

        def bc(ap):
            return ap

    nc = bacc.Bacc("TRN2", target_bir_lowering=False, debug=False)
    encT = nc.declare_dram_parameter("encT", [BL, D, S], f32, isOutput=False)
    enc = nc.declare_dram_parameter("enc", [S, BL, D], f32, isOutput=False)
    uaT = nc.declare_dram_parameter("uaT", [D, H], mdt, isOutput=False)
    waT = nc.declare_dram_parameter("waT", [H, H], f32, isOutput=False)
    hidT = nc.declare_dram_parameter("hidT", [H, BL], f32, isOutput=False)
    va2 = nc.declare_dram_parameter("va2", [P, KH], mdt, isOutput=False)
    ones = nc.declare_dram_parameter("ones", [1, 1], mdt, isOutput=False)
    out_ctx = nc.declare_dram_parameter("out_ctx", [BL, D], f32, isOutput=True)
    out_attn = nc.declare_dram_parameter("out_attn", [BL, S], f32, isOutput=True)

    AF = mybir.ActivationFunctionType

    with tile.TileContext(nc) as tc:
        with (
            tc.tile_pool(name="weights", bufs=1) as wpool,
            tc.tile_pool(name="wa", bufs=3) as wapool,
            tc.tile_pool(name="et", bufs=3) as etpool,
            tc.tile_pool(name="etm", bufs=2) as etmpool,
            tc.tile_pool(name="en", bufs=3) as enpool,
            tc.tile_pool(name="sm", bufs=2) as smpool,
            tc.tile_pool(name="ct", bufs=3) as ctpool,
            tc.tile_pool(name="misc", bufs=2) as miscpool,
            tc.tile_pool(name="pe", bufs=3, space="PSUM") as pe_psum,
            tc.tile_pool(name="small", bufs=3, space="PSUM") as small_psum,
            tc.tile_pool(name="pt", bufs=2, space="PSUM") as pt_psum,
        ):
            # ---- persistent weights ----
            ua_sb = wpool.tile([P, KD, H], mdt)
            nc.sync.dma_start(ua_sb[:], uaT[:].rearrange("(ko p) h -> p ko h", p=P))
            va_sb = wpool.tile([P, KH], mdt)
            nc.sync.dma_start(va_sb[:], va2[:])
            ones_sb = wpool.tile([1, 1], mdt)
            nc.sync.dma_start(ones_sb[:], ones[:])
            hid_sb = wpool.tile([P, KH, BL], f32)
            nc.sync.dma_start(hid_sb[:], hidT[:].rearrange("(ko p) b -> p ko b", p=P))
            hp_sb = wpool.tile([P, KH, BL], f32)

            # ---- h_proj (transposed): hp[h, b] = sum_k Wa.T[k, h] * hidden.T[k, b]
            for m in range(KH):
                php = small_psum.tile([P, BL], f32, tag="small")
                for k in range(KH):
                    wa_t = wapool.tile([P, P], f32, tag="wa")
                    nc.sync.dma_start(
                        wa_t[:], waT[k * P : (k + 1) * P, m * P : (m + 1) * P]
                    )
                    nc.tensor.matmul(
                        php[:],
                        lhsT=wa_t[:],
                        rhs=hid_sb[:, k, :],
                        start=(k == 0),
                        stop=(k == KH - 1),
                    )
                nc.vector.tensor_copy(hp_sb[:, m, :], php[:])

            for b in range(BL):
                # ---- e_proj / energy / scores ----
                scores = smpool.tile([1, S], f32, tag="scores")
                for n in range(NS):
                    ssl = slice(n * NCH, (n + 1) * NCH)
                    ets = []
                    for h in range(2):
                        et = etpool.tile([P, KD // 2, NCH], f32, tag="et")
                        nc.sync.dma_start(
                            et[:],
                            encT[b, h * (D // 2) : (h + 1) * (D // 2), ssl].rearrange(
                                "(ko p) s -> p ko s", p=P
                            ),
                        )
                        ets.append(et)
                    if mm_dt == "bf16":
                        etm = etmpool.tile([P, KD, NCH], mdt, tag="etm")
                        nc.vector.tensor_copy(etm[:, 0 : KD // 2, :], ets[0][:])
                        nc.vector.tensor_copy(etm[:, KD // 2 : KD, :], ets[1][:])

                        def rhs_k(k):
                            return etm[:, k, :]
                    else:

                        def rhs_k(k, ets=ets):
                            return ets[k // (KD // 2)][:, k % (KD // 2), :]

                    scp = small_psum.tile([1, NCH], f32, tag="small")
                    for m in range(KH):
                        ps = pe_psum.tile([P, NCH], f32, tag="pe")
                        for k in range(KD):
                            nc.tensor.matmul(
                                ps[:],
                                lhsT=bc(ua_sb[:, k, m * P : (m + 1) * P]),
                                rhs=bc(rhs_k(k)),
                                start=(k == 0),
                                stop=(k == KD - 1),
                            )
                        en = enpool.tile([P, NCH], mdt, tag="en")
                        nc.scalar.activation(
                            en[:], ps[:], AF.Relu, bias=hp_sb[:, m, b : b + 1]
                        )
                        nc.tensor.matmul(
                            scp[:],
                            lhsT=bc(va_sb[:, m : m + 1]),
                            rhs=bc(en[:]),
                            start=(m == 0),
                            stop=(m == KH - 1),
                        )
                    nc.vector.tensor_copy(scores[:, ssl], scp[:])

                # ---- softmax (scores are O(+-10); exp without max-shift is safe)
                ssum = miscpool.tile([1, 1], f32, tag="ssum")
                nc.scalar.activation(scores[:], scores[:], AF.Exp, accum_out=ssum[:])
                sinv = miscpool.tile([1, 1], f32, tag="sinv")
                nc.vector.reciprocal(sinv[:], ssum[:])
                nc.vector.tensor_scalar_mul(scores[:], scores[:], sinv[:])
                nc.sync.dma_start(out_attn[b : b + 1, :], scores[:])
                if mm_dt == "bf16":
                    attn_m = smpool.tile([1, S], mdt, tag="attnm")
                    nc.vector.tensor_copy(attn_m[:], scores[:])
                else:
                    attn_m = scores

                # ---- transpose attn row -> column tiles via K=1 matmul with ones
                pt = pt_psum.tile([P, KS], f32, tag="pt")
                for k in range(KS):
                    nc.tensor.matmul(
                        pt[:, k : k + 1],
                        lhsT=bc(attn_m[:, k * P : (k + 1) * P]),
                        rhs=bc(ones_sb[:]),
                        start=True,
                        stop=True,
                    )
                attnT = miscpool.tile([P, KS], mdt, tag="attnT")
                nc.vector.tensor_copy(attnT[:], pt[:])

                # ---- context: ctx[d] = sum_s attn[s] * enc[s, d]
                for nd in range(ND):
                    dsl = slice(nd * NCH, (nd + 1) * NCH)
                    cps = small_psum.tile([1, NCH], f32, tag="small")
                    for kg in range(KS // 4):
                        est = ctpool.tile([P, 4, NCH], f32, tag="est")
                        nc.sync.dma_start(
                            est[:],
                            enc[kg * 512 : (kg + 1) * 512, b, dsl].rearrange(
                                "(ko p) d -> p ko d", p=P
                            ),
                        )
                        if mm_dt == "bf16":
                            estm = ctpool.tile([P, 4, NCH], mdt, tag="estm")
                            nc.scalar.activation(estm[:], est[:], AF.Copy)
                        else:
                            estm = est
                        for kk in range(4):
                            k = kg * 4 + kk
                            nc.tensor.matmul(
                                cps[:],
                                lhsT=bc(attnT[:, k : k + 1]),
                                rhs=bc(estm[:, kk, :]),
                                start=(k == 0),
                                stop=(k == KS - 1),
                            )
                    crow = miscpool.tile([1, NCH], f32, tag="crow")
                    nc.vector.tensor_copy(crow[:], cps[:])
                    nc.sync.dma_start(out_ctx[b : b + 1, dsl], crow[:])

    nc.compile()
    return nc


def _get_nc(mm_dt: str):
    if mm_dt not in _CACHE:
        _CACHE[mm_dt] = _build(mm_dt)
    return _CACHE[mm_dt]


def kernel(hidden, encoder_outputs, Wa, Ua, Va, _trace=False):
    mm_dt = MM_DT
    nc = _get_nc(mm_dt)

    bf16 = ml_dtypes.bfloat16
    wdt = bf16 if mm_dt == "bf16" else np.float32

    # host-side layout prep (sharding)
    encT_all = np.ascontiguousarray(
        encoder_outputs.transpose(1, 2, 0)
    )  # [B, D, S]
    uaT_np = np.ascontiguousarray(Ua.T).astype(wdt)  # [D, H]
    waT_np = np.ascontiguousarray(Wa.T).astype(np.float32)  # [H, H]
    va2_np = np.ascontiguousarray(Va[0].reshape(KH, P).T).astype(wdt)  # [P, KH]
    ones_np = np.ones((1, 1), dtype=wdt)

    in_maps = []
    for c in range(NCORES):
        b0 = c * BL
        in_maps.append(
            {
                "encT": encT_all[b0 : b0 + BL],
                "enc": np.ascontiguousarray(encoder_outputs[:, b0 : b0 + BL, :]),
                "uaT": uaT_np,
                "waT": waT_np,
                "hidT": np.ascontiguousarray(hidden[b0 : b0 + BL].T),
                "va2": va2_np,
                "ones": ones_np,
            }
        )

    res = bass_utils.run_bass_kernel_spmd(
        nc, in_maps, core_ids=list(range(NCORES)), trace=_trace
    )

    ctx = np.concatenate([res.results[c]["out_ctx"] for c in range(NCORES)], axis=0)
    attn = np.concatenate([res.results[c]["out_attn"] for c in range(NCORES)], axis=0)
    out = (ctx.reshape(B, 1, D).astype(np.float32), attn.astype(np.float32))
    if _trace:
        return out, res
    return out


# revision 6
# speedup vs baseline: 1.1722x; 1.1722x over previous
"""Bahdanau attention on 8 Trainium2 NeuronCores.

Problem (full shapes): hidden [32,1024], encoder_outputs [2048,32,2048],
Wa [1024,1024], Ua [1024,2048], Va [1,1024].

reference:
    enc    = encoder_outputs.transpose(1,0,2)        # [B,S,2H]
    h_proj = hidden @ Wa.T                           # [B,H]
    e_proj = einsum('bsd,hd->bsh', enc, Ua)          # [B,S,H]
    energy = relu(h_proj[:,None,:] + e_proj)         # [B,S,H]
    scores = einsum('bsh,h->bs', energy, Va[0])      # [B,S]
    attn   = softmax(scores, axis=-1)                # [B,S]
    ctx    = einsum('bs,bsd->bd', attn, enc)[:,None] # [B,1,2H]
    return (ctx, attn)

Strategy: data-parallel over batch (4 batches/core).  Per batch the
dominant matmul is enc_b @ Ua.T, computed transposed as e_projT[h,s] so
the relu+h_proj bias is a per-partition scalar on the scalar engine and
the Va contraction is a K=H matmul.  Softmax is computed unnormalized
(scores are O(+-10) so exp needs no max shift); the context is
accumulated flash-style per s-chunk with unnormalized exp weights and
scaled by 1/Z at the end, so its enc[s,d] streaming overlaps the main
matmul instead of forming a DMA-bound tail.  Host pre-transposes
encoder_outputs so contraction dims land on SBUF partitions.
"""

import os
import sys

sys.path.insert(0, "/opt/trn_rl_repo")

import numpy as np
import ml_dtypes

import concourse.bacc as bacc
import concourse.tile as tile
import concourse.mybir as mybir
from concourse import bass_utils

B, S, H = 32, 2048, 1024
D = 2 * H
NCORES = 8
BL = B // NCORES  # batches per core
P = 128
NCH = 512          # matmul moving free-dim chunk (one PSUM bank)
KD = D // P        # 16 contraction tiles over D (e_proj)
KH = H // P        # 8 contraction tiles over H (scores)
NS = S // NCH      # 4 s-chunks
ND = D // NCH      # 4 d-chunks
KC = NCH // P      # 4 s k-tiles per chunk (context)

# "bf16" | "f32" | "f32r"
MM_DT = os.environ.get("BAH_MM_DT", "bf16")

_CACHE = {}


def _build(mm_dt: str):
    f32 = mybir.dt.float32
    if mm_dt == "bf16":
        mdt = mybir.dt.bfloat16

        def bc(ap):
            return ap
    elif mm_dt == "f32r":
        mdt = f32

        def bc(ap):
            return ap.bitcast(mybir.dt.float32r)
    else:
        mdt = f32

        def bc(ap):
            return ap

    nc = bacc.Bacc("TRN2", target_bir_lowering=False, debug=False)
    encT = nc.declare_dram_parameter("encT", [BL, D, S], f32, isOutput=False)
    enc = nc.declare_dram_parameter("enc", [S, BL, D], f32, isOutput=False)
    uaT = nc.declare_dram_parameter("uaT", [D, H], mdt, isOutput=False)
    waT = nc.declare_dram_parameter("waT", [H, H], f32, isOutput=False)
    hidT = nc.declare_dram_parameter("hidT", [H, BL], f32, isOutput=False)
    va2 = nc.declare_dram_parameter("va2", [P, KH], mdt, isOutput=False)
    ones = nc.declare_dram_parameter("ones", [1, 1], mdt, isOutput=False)
    out_ctx = nc.declare_dram_parameter("out_ctx", [BL, D], f32, isOutput=True)
    out_attn = nc.declare_dram_parameter("out_attn", [BL, S], f32, isOutput=True)

    AF = mybir.ActivationFunctionType

    with tile.TileContext(nc) as tc:
        with (
            tc.tile_pool(name="weights", bufs=1) as wpool,
            tc.tile_pool(name="wa", bufs=6) as wapool,
            tc.tile_pool(name="et", bufs=3) as etpool,
            tc.tile_pool(name="etm", bufs=2) as etmpool,
            tc.tile_pool(name="en", bufs=3) as enpool,
            tc.tile_pool(name="sm", bufs=2) as smpool,
            tc.tile_pool(name="ct", bufs=3) as ctpool,
            tc.tile_pool(name="misc", bufs=2) as miscpool,
            tc.tile_pool(name="pe", bufs=2, space="PSUM") as pe_psum,
            tc.tile_pool(name="small", bufs=2, space="PSUM") as small_psum,
            tc.tile_pool(name="cx", bufs=4, space="PSUM") as cx_psum,
        ):
            # ---- persistent weights (ua split per k-tile so the first
            # matmul only waits for one 256KB slice) ----
            ua_sb = wpool.tile([P, KD, H], mdt)
            uaT_r = uaT[:].rearrange("(ko p) h -> p ko h", p=P)
            for k in range(KD):
                nc.sync.dma_start(ua_sb[:, k : k + 1, :], uaT_r[:, k : k + 1, :])
            va_sb = wpool.tile([P, KH], mdt)
            nc.sync.dma_start(va_sb[:], va2[:])
            ones_sb = wpool.tile([1, 1], mdt)
            nc.sync.dma_start(ones_sb[:], ones[:])
            hid_sb = wpool.tile([P, KH, BL], f32)
            nc.sync.dma_start(hid_sb[:], hidT[:].rearrange("(ko p) b -> p ko b", p=P))
            hp_sb = wpool.tile([P, KH, BL], f32)

            def emit_hp(m):
                # hp[h, b] = sum_k Wa.T[k, h] * hidden.T[k, b] for h-tile m
                php = small_psum.tile([P, BL], f32, tag="sp", name="php")
                for k in range(KH):
                    wa_t = wapool.tile([P, P], f32, tag="wa")
                    nc.sync.dma_start(
                        wa_t[:], waT[k * P : (k + 1) * P, m * P : (m + 1) * P]
                    )
                    nc.tensor.matmul(
                        php[:],
                        lhsT=wa_t[:],
                        rhs=hid_sb[:, k, :],
                        start=(k == 0),
                        stop=(k == KH - 1),
                    )
                nc.vector.tensor_copy(hp_sb[:, m, :], php[:])

            for b in range(BL):
                scores = smpool.tile([1, S], f32, tag="scores")
                zrow = miscpool.tile([1, NS], f32, tag="zrow")
                cps = [
                    cx_psum.tile([1, NCH], f32, tag="cx", name=f"cps{nd}")
                    for nd in range(ND)
                ]
                for n in range(NS):
                    ssl = slice(n * NCH, (n + 1) * NCH)
                    ets = []
                    for h in range(2):
                        et = etpool.tile([P, KD // 2, NCH], f32, tag="et")
                        nc.sync.dma_start(
                            et[:],
                            encT[b, h * (D // 2) : (h + 1) * (D // 2), ssl].rearrange(
                                "(ko p) s -> p ko s", p=P
                            ),
                        )
                        ets.append(et)
                    if mm_dt == "bf16":
                        etm = etmpool.tile([P, KD, NCH], mdt, tag="etm")
                        nc.vector.tensor_copy(etm[:, 0 : KD // 2, :], ets[0][:])
                        nc.vector.tensor_copy(etm[:, KD // 2 : KD, :], ets[1][:])

                        def rhs_k(k):
                            return etm[:, k, :]
                    else:

                        def rhs_k(k, ets=ets):
                            return ets[k // (KD // 2)][:, k % (KD // 2), :]

                    scp = small_psum.tile([1, NCH], f32, tag="sp", name="scp")
                    for m in range(KH):
                        if b == 0 and n == 0:
                            emit_hp(m)
                        ps = pe_psum.tile([P, NCH], f32, tag="pe")
                        for k in range(KD):
                            nc.tensor.matmul(
                                ps[:],
                                lhsT=bc(ua_sb[:, k, m * P : (m + 1) * P]),
                                rhs=bc(rhs_k(k)),
                                start=(k == 0),
                                stop=(k == KD - 1),
                            )
                        en = enpool.tile([P, NCH], mdt, tag="en")
                        nc.scalar.activation(
                            en[:], ps[:], AF.Relu, bias=hp_sb[:, m, b : b + 1]
                        )
                        nc.tensor.matmul(
                            scp[:],
                            lhsT=bc(va_sb[:, m : m + 1]),
                            rhs=bc(en[:]),
                            start=(m == 0),
                            stop=(m == KH - 1),
                        )
                    # exp (unnormalized softmax numerator) + running Z
                    nc.vector.tensor_copy(scores[:, ssl], scp[:])
                    nc.scalar.activation(
                        scores[:, ssl],
                        scores[:, ssl],
                        AF.Exp,
                        accum_out=zrow[:, n : n + 1],
                    )
                    if mm_dt == "bf16":
                        exp_m = smpool.tile([1, NCH], mdt, tag="expm")
                        nc.vector.tensor_copy(exp_m[:], scores[:, ssl])
                    else:
                        exp_m = scores[:, ssl]
                    # transpose exp chunk into column tiles via K=1 matmul
                    pt = small_psum.tile([P, KC], f32, tag="sp", name="pt")
                    for kk in range(KC):
                        nc.tensor.matmul(
                            pt[:, kk : kk + 1],
                            lhsT=bc(exp_m[:, kk * P : (kk + 1) * P]),
                            rhs=bc(ones_sb[:]),
                            start=True,
                            stop=True,
                        )
                    attnT = miscpool.tile([P, KC], mdt, tag="attnT")
                    nc.vector.tensor_copy(attnT[:], pt[:])
                    # unnormalized context accumulation for this s-chunk
                    for nd in range(ND):
                        dsl = slice(nd * NCH, (nd + 1) * NCH)
                        est = ctpool.tile([P, KC, NCH], f32, tag="est")
                        nc.sync.dma_start(
                            est[:],
                            enc[n * NCH : (n + 1) * NCH, b, dsl].rearrange(
                                "(ko p) d -> p ko d", p=P
                            ),
                        )
                        if mm_dt == "bf16":
                            estm = ctpool.tile([P, KC, NCH], mdt, tag="estm")
                            nc.scalar.activation(estm[:], est[:], AF.Copy)
                        else:
                            estm = est
                        for kk in range(KC):
                            nc.tensor.matmul(
                                cps[nd][:],
                                lhsT=bc(attnT[:, kk : kk + 1]),
                                rhs=bc(estm[:, kk, :]),
                                start=(n == 0 and kk == 0),
                                stop=(n == NS - 1 and kk == KC - 1),
                            )
                # ---- normalize: Z, attn out, ctx out ----
                zsum = miscpool.tile([1, 1], f32, tag="zsum")
                nc.vector.reduce_sum(zsum[:], zrow[:], axis=mybir.AxisListType.X)
                sinv = miscpool.tile([1, 1], f32, tag="sinv")
                nc.vector.reciprocal(sinv[:], zsum[:])
                nc.vector.tensor_scalar_mul(scores[:], scores[:], sinv[:])
                nc.sync.dma_start(out_attn[b : b + 1, :], scores[:])
                for nd in range(ND):
                    crow = miscpool.tile([1, NCH], f32, tag="crow")
                    nc.vector.tensor_scalar_mul(crow[:], cps[nd][:], sinv[:])
                    nc.sync.dma_start(
                        out_ctx[b : b + 1, nd * NCH : (nd + 1) * NCH], crow[:]
                    )

    nc.compile()
    return nc


def _get_nc(mm_dt: str):
    if mm_dt not in _CACHE:
        _CACHE[mm_dt] = _build(mm_dt)
    return _CACHE[mm_dt]


def kernel(hidden, encoder_outputs, Wa, Ua, Va, _trace=False):
    mm_dt = MM_DT
    nc = _get_nc(mm_dt)

    wdt = ml_dtypes.bfloat16 if mm_dt == "bf16" else np.float32

    # host-side layout prep (sharding)
    encT_all = np.ascontiguousarray(encoder_outputs.transpose(1, 2, 0))  # [B, D, S]
    uaT_np = np.ascontiguousarray(Ua.T).astype(wdt)  # [D, H]
    waT_np = np.ascontiguousarray(Wa.T).astype(np.float32)  # [H, H]
    va2_np = np.ascontiguousarray(Va[0].reshape(KH, P).T).astype(wdt)  # [P, KH]
    ones_np = np.ones((1, 1), dtype=wdt)

    in_maps = []
    for c in range(NCORES):
        b0 = c * BL
        in_maps.append(
            {
                "encT": encT_all[b0 : b0 + BL],
                "enc": np.ascontiguousarray(encoder_outputs[:, b0 : b0 + BL, :]),
                "uaT": uaT_np,
                "waT": waT_np,
                "hidT": np.ascontiguousarray(hidden[b0 : b0 + BL].T),
                "va2": va2_np,
                "ones": ones_np,
            }
        )

    res = bass_utils.run_bass_kernel_spmd(
        nc, in_maps, core_ids=list(range(NCORES)), trace=_trace
    )

    ctx = np.concatenate([res.results[c]["out_ctx"] for c in range(NCORES)], axis=0)
    attn = np.concatenate([res.results[c]["out_attn"] for c in range(NCORES)], axis=0)
    out = (ctx.reshape(B, 1, D).astype(np.float32), attn.astype(np.float32))
    if _trace:
        return out, res
    return out
